# revision 10
# baseline (speedup 1.0000x reference)
"""Two-layer GAT on 8 Trainium2 NeuronCores.

Strategy: shard by destination-node range. Edges (with self-loops) are sorted
by destination on the host; core c owns destinations [c*NPC, (c+1)*NPC) and
computes those output rows entirely locally, so per-destination softmax and
aggregation need no cross-core reduction. The only collectives are two small
AllGathers of the packed node features (h after layer-1 GEMM, h2 after
layer-2 GEMM) so every core can gather arbitrary source rows.

Per core, per layer the message passing runs over fixed-size edge tiles of
128 edges grouped into 128-destination windows:
  - dma_gather fetches h[src] rows (bf16, 256B-multiple rows) for a whole
    window in one instruction,
  - a one-hot matrix S[e, d] built with is_equal(iota) scatters
    exp-weighted messages into PSUM via the TensorEngine,
  - S^T (via a K=1 replicate-matmul + is_equal) pulls a_d[dst] to edge slots.
Softmax normalizers ride as extra columns of the same scatter matmul.
"""
import sys

sys.path.insert(0, "/opt/trn_rl_repo")
import numpy as np
import ml_dtypes

import concourse.bass as bass
import concourse.bacc as bacc
import concourse.mybir as mybir
import concourse.tile as tile
from concourse.bass_utils import run_bass_kernel_spmd
from concourse.masks import make_identity

BF16 = mybir.dt.bfloat16
F32 = mybir.dt.float32
I16 = mybir.dt.int16
nbf16 = ml_dtypes.bfloat16

NCORES = 8
NEG = 0.2
P = 128


def _to_bf(a):
    return np.ascontiguousarray(np.asarray(a, dtype=np.float32).astype(nbf16))


def _build_program(N, IN_C, F1, H, C, OUT_C, NPAD, T, TPW, W, TG, trace_label=None):
    GC = 8
    KT1 = IN_C // P
    KT2 = F1 // P
    F1C = F1 // P
    OC2 = OUT_C + 2
    R2 = 128  # layer-2 packed row width (bf16) -> 256B
    NROWS = NCORES * NPAD

    nc = bacc.Bacc(num_devices=NCORES, dynamic_dma_scratch_size=65536)

    xT_d = nc.dram_tensor("xT", [IN_C, NPAD], BF16, kind="ExternalInput")
    w1_d = nc.dram_tensor("w1b", [IN_C, F1], BF16, kind="ExternalInput")
    attsrc_d = nc.dram_tensor("attsrc", [P, F1], BF16, kind="ExternalInput")
    attdst_d = nc.dram_tensor("attdst", [P, F1], BF16, kind="ExternalInput")
    w2e_d = nc.dram_tensor("w2e", [F1, OC2], BF16, kind="ExternalInput")
    b1_d = nc.dram_tensor("b1m", [P, F1], F32, kind="ExternalInput")
    b2_d = nc.dram_tensor("b2m", [P, OUT_C], F32, kind="ExternalInput")
    idx_d = nc.dram_tensor("idx", [P, TG * 8], I16, kind="ExternalInput")
    dstp_d = nc.dram_tensor("dstp", [P, T], BF16, kind="ExternalInput")
    TF3 = ((T + 2) // 3) * P
    dstf_d = nc.dram_tensor("dstf", [P, TF3], BF16, kind="ExternalInput")
    iotaf_d = nc.dram_tensor("iotaf", [P, P], BF16, kind="ExternalInput")
    iotap_d = nc.dram_tensor("iotap", [P, 1], BF16, kind="ExternalInput")
    out_d = nc.dram_tensor("out", [NPAD, OUT_C], F32, kind="ExternalOutput")

    hloc_d = nc.dram_tensor("hloc", [NPAD, F1], BF16)
    hfull_d = nc.dram_tensor("hfull", [NROWS, F1], BF16, addr_space="Shared")
    h2loc_d = nc.dram_tensor("h2loc", [NPAD, R2], BF16)
    h2full_d = nc.dram_tensor("h2full", [NROWS, R2], BF16, addr_space="Shared")

    rg = [list(range(NCORES))]

    with tile.TileContext(nc) as tc:
        with (
            tc.tile_pool(name="const", bufs=1) as cp,
            tc.tile_pool(name="persist", bufs=1) as pp,
        ):
            iotaf = cp.tile([P, P], BF16)
            nc.sync.dma_start(out=iotaf[:], in_=iotaf_d[:, :])
            iotap = cp.tile([P, 1], BF16)
            nc.sync.dma_start(out=iotap[:], in_=iotap_d[:, :])
            ones1 = cp.tile([P, P], BF16)
            nc.gpsimd.memset(ones1[:], 1.0)
            attsrc = cp.tile([P, F1], BF16)
            nc.sync.dma_start(out=attsrc[:], in_=attsrc_d[:, :])
            attdst = cp.tile([P, F1], BF16)
            nc.sync.dma_start(out=attdst[:], in_=attdst_d[:, :])
            b1m = cp.tile([P, F1], F32)
            nc.sync.dma_start(out=b1m[:], in_=b1_d[:, :])
            b2m = cp.tile([P, OUT_C], F32)
            nc.sync.dma_start(out=b2m[:], in_=b2_d[:, :])
            w1sb = cp.tile([P, KT1 * F1], BF16)
            for k in range(KT1):
                nc.sync.dma_start(out=w1sb[:, k * F1:(k + 1) * F1], in_=w1_d[k * P:(k + 1) * P, :])
            w2esb = cp.tile([P, KT2 * OC2], BF16)
            for k in range(KT2):
                nc.sync.dma_start(out=w2esb[:, k * OC2:(k + 1) * OC2], in_=w2e_d[k * P:(k + 1) * P, :])
            idxsb = cp.tile([P, TG * 8], I16)
            nc.sync.dma_start(out=idxsb[:], in_=idx_d[:, :])
            dstp = cp.tile([P, T], BF16)
            nc.sync.dma_start(out=dstp[:], in_=dstp_d[:, :])
            dstf = cp.tile([P, TF3], BF16)
            nc.sync.dma_start(out=dstf[:], in_=dstf_d[:, :])
            ident = cp.tile([P, P], BF16)
            make_identity(nc, ident[:])

            xT = pp.tile([P, KT1 * NPAD], BF16)
            for k in range(KT1):
                nc.sync.dma_start(out=xT[:, k * NPAD:(k + 1) * NPAD], in_=xT_d[k * P:(k + 1) * P, :])
            adw = pp.tile([P, W * H], BF16)
            ad2w = pp.tile([P, W], BF16)
            h1T = pp.tile([P, F1C * NPAD], BF16)

            # ---------------- Phase A: GEMM1 + a_d ----------------
            with (
                tc.tile_pool(name="psA", bufs=2, space="PSUM") as psA,
                tc.tile_pool(name="wpA", bufs=3) as wpA,
            ):
                for m in range(W):
                    ph = psA.tile([P, F1], F32, tag="ph")
                    for k in range(KT1):
                        nc.tensor.matmul(
                            ph[:],
                            lhsT=xT[:, k * NPAD + m * P: k * NPAD + (m + 1) * P],
                            rhs=w1sb[:, k * F1:(k + 1) * F1],
                            start=(k == 0), stop=(k == KT1 - 1),
                        )
                    hf = wpA.tile([P, F1], F32, tag="hf")
                    nc.vector.tensor_add(out=hf[:], in0=ph[:], in1=b1m[:])
                    prod = wpA.tile([P, F1], BF16, tag="prod")
                    nc.vector.tensor_mul(out=prod[:], in0=hf[:], in1=attdst[:])
                    adf = wpA.tile([P, H], F32, tag="adf")
                    nc.vector.tensor_reduce(
                        out=adf[:], in_=prod[:].rearrange("p (h c) -> p h c", c=C),
                        axis=mybir.AxisListType.X, op=mybir.AluOpType.add,
                    )
                    nc.vector.tensor_copy(out=adw[:, m * H:(m + 1) * H], in_=adf[:])
                    hb = wpA.tile([P, F1], BF16, tag="hb")
                    nc.vector.tensor_copy(out=hb[:], in_=hf[:])
                    nc.sync.dma_start(out=hloc_d[m * P:(m + 1) * P, :], in_=hb[:])

            nc.gpsimd.collective_compute(
                "AllGather", mybir.AluOpType.bypass, replica_groups=rg,
                ins=[hloc_d.ap()], outs=[hfull_d.ap()],
            )

            # ---------------- Phase B: layer-1 message passing ----------------
            with (
                tc.tile_pool(name="gpB", bufs=2) as gpB,
                tc.tile_pool(name="psAcc", bufs=2, space="PSUM") as psAcc,
                tc.tile_pool(name="psD", bufs=2, space="PSUM") as psD,
                tc.tile_pool(name="psE", bufs=2, space="PSUM") as psE,
                tc.tile_pool(name="psT", bufs=2, space="PSUM") as psT,
                tc.tile_pool(name="wpB", bufs=3) as wpB,
            ):
                chunk_tiles = {}
                for w in range(W):
                    acc = psAcc.tile([P, F1 + H], F32, tag="acc")
                    for tl in range(TPW):
                        t = w * TPW + tl
                        c, slot = divmod(t, GC)
                        if c not in chunk_tiles:
                            gt = gpB.tile([P, GC * F1], BF16, tag="gath")
                            nc.gpsimd.dma_gather(
                                out_ap=gt[:].rearrange("p (t f) -> p t f", f=F1),
                                in_ap=hfull_d.ap(),
                                idxs_ap=idxsb[:, c * GC * 8:(c + 1) * GC * 8],
                                num_idxs=GC * P, num_idxs_reg=GC * P, elem_size=F1,
                            )
                            chunk_tiles[c] = gt
                        hsrc = chunk_tiles[c][:, slot * F1:(slot + 1) * F1]
                        S = wpB.tile([P, P], BF16, tag="S")
                        nc.vector.tensor_tensor(
                            out=S[:], in0=dstp[:, t:t + 1].to_broadcast([P, P]),
                            in1=iotaf[:], op=mybir.AluOpType.is_equal,
                        )
                        bp = (t % 3) * 32
                        dmat = psD.tile([P, P], F32, tag="dmat")
                        nc.tensor.matmul(
                            dmat[:], lhsT=ones1[bp:bp + 1, :],
                            rhs=dstf[bp:bp + 1, (t // 3) * P:(t // 3 + 1) * P],
                            start=True, stop=True,
                        )
                        ST = wpB.tile([P, P], BF16, tag="ST")
                        nc.vector.tensor_tensor(
                            out=ST[:], in0=iotap[:, 0:1].to_broadcast([P, P]),
                            in1=dmat[:], op=mybir.AluOpType.is_equal,
                        )
                        ade = psE.tile([P, H], F32, tag="ade")
                        nc.tensor.matmul(
                            ade[:], lhsT=ST[:], rhs=adw[:, w * H:(w + 1) * H],
                            start=True, stop=True,
                        )
                        prod = wpB.tile([P, F1], BF16, tag="prodB")
                        nc.vector.tensor_mul(out=prod[:], in0=hsrc, in1=attsrc[:])
                        a_s = wpB.tile([P, H], F32, tag="a_s")
                        nc.vector.tensor_reduce(
                            out=a_s[:], in_=prod[:].rearrange("p (h c) -> p h c", c=C),
                            axis=mybir.AxisListType.X, op=mybir.AluOpType.add,
                        )
                        esc = wpB.tile([P, H], F32, tag="esc")
                        nc.vector.tensor_add(out=esc[:], in0=a_s[:], in1=ade[:])
                        elr = wpB.tile([P, H], F32, tag="elr")
                        nc.vector.scalar_tensor_tensor(
                            out=elr[:], in0=esc[:], scalar=NEG, in1=esc[:],
                            op0=mybir.AluOpType.mult, op1=mybir.AluOpType.max,
                        )
                        rhs = wpB.tile([P, F1 + H], BF16, tag="rhs")
                        nc.scalar.activation(rhs[:, F1:F1 + H], elr[:], mybir.ActivationFunctionType.Exp)
                        nc.vector.tensor_mul(
                            out=rhs[:, :F1].rearrange("p (h c) -> p h c", c=C),
                            in0=hsrc.rearrange("p (h c) -> p h c", c=C),
                            in1=rhs[:, F1:F1 + H].rearrange("p (h c) -> p h c", c=1).to_broadcast([P, H, C]),
                        )
                        nc.tensor.matmul(
                            acc[:], lhsT=S[:], rhs=rhs[:],
                            start=(tl == 0), stop=(tl == TPW - 1),
                        )
                    zr = wpB.tile([P, H], F32, tag="zr")
                    nc.vector.reciprocal(zr[:], acc[:, F1:F1 + H])
                    o1 = wpB.tile([P, F1], F32, tag="o1")
                    nc.vector.tensor_mul(
                        out=o1[:].rearrange("p (h c) -> p h c", c=C),
                        in0=acc[:, :F1].rearrange("p (h c) -> p h c", c=C),
                        in1=zr[:].rearrange("p (h c) -> p h c", c=1).to_broadcast([P, H, C]),
                    )
                    nc.vector.tensor_add(out=o1[:], in0=o1[:], in1=b1m[:])
                    # ELU = (exp(min(x,0)) - 1) + max(x, 0)
                    mn = wpB.tile([P, F1], F32, tag="mn")
                    nc.vector.tensor_scalar_min(out=mn[:], in0=o1[:], scalar1=0.0)
                    em = wpB.tile([P, F1], F32, tag="em")
                    nc.scalar.activation(em[:], mn[:], mybir.ActivationFunctionType.Exp)
                    mx = wpB.tile([P, F1], F32, tag="mx")
                    nc.vector.tensor_scalar_max(out=mx[:], in0=o1[:], scalar1=0.0)
                    h1 = wpB.tile([P, F1], BF16, tag="h1")
                    nc.vector.scalar_tensor_tensor(
                        out=h1[:], in0=em[:], scalar=-1.0, in1=mx[:],
                        op0=mybir.AluOpType.add, op1=mybir.AluOpType.add,
                    )
                    for fc in range(F1C):
                        tp = psT.tile([P, P], BF16, tag="tp")
                        nc.tensor.transpose(tp[:], h1[:, fc * P:(fc + 1) * P], ident[:])
                        nc.vector.tensor_copy(
                            out=h1T[:, fc * NPAD + w * P: fc * NPAD + (w + 1) * P], in_=tp[:],
                        )

            # ---------------- Phase C: GEMM2 + pack ----------------
            with (
                tc.tile_pool(name="psC", bufs=2, space="PSUM") as psC,
                tc.tile_pool(name="wpC", bufs=3) as wpC,
            ):
                for m in range(W):
                    p2 = psC.tile([P, OC2], F32, tag="p2")
                    for k in range(KT2):
                        nc.tensor.matmul(
                            p2[:],
                            lhsT=h1T[:, k * NPAD + m * P: k * NPAD + (m + 1) * P],
                            rhs=w2esb[:, k * OC2:(k + 1) * OC2],
                            start=(k == 0), stop=(k == KT2 - 1),
                        )
                    h2r = wpC.tile([P, R2], BF16, tag="h2r")
                    nc.vector.memset(h2r[:, OC2:], 0.0)
                    nc.vector.tensor_copy(out=h2r[:, :OC2], in_=p2[:])
                    nc.vector.tensor_copy(out=ad2w[:, m:m + 1], in_=p2[:, OC2 - 1:OC2])
                    nc.sync.dma_start(out=h2loc_d[m * P:(m + 1) * P, :], in_=h2r[:])

            nc.gpsimd.collective_compute(
                "AllGather", mybir.AluOpType.bypass, replica_groups=rg,
                ins=[h2loc_d.ap()], outs=[h2full_d.ap()],
            )

            # ---------------- Phase D: layer-2 message passing ----------------
            with (
                tc.tile_pool(name="gpD", bufs=2) as gpD,
                tc.tile_pool(name="psAcc2", bufs=2, space="PSUM") as psAcc2,
                tc.tile_pool(name="psD2", bufs=2, space="PSUM") as psD2,
                tc.tile_pool(name="psE2", bufs=2, space="PSUM") as psE2,
                tc.tile_pool(name="wpD", bufs=3) as wpD,
            ):
                OC1 = OUT_C + 1
                chunk2 = {}
                for w in range(W):
                    acc2 = psAcc2.tile([P, OC1], F32, tag="acc2")
                    for tl in range(TPW):
                        t = w * TPW + tl
                        c2, slot2 = divmod(t, GC)
                        if c2 not in chunk2:
                            g2 = gpD.tile([P, GC * R2], BF16, tag="gath2")
                            nc.gpsimd.dma_gather(
                                out_ap=g2[:].rearrange("p (t f) -> p t f", f=R2),
                                in_ap=h2full_d.ap(),
                                idxs_ap=idxsb[:, c2 * GC * 8:(c2 + 1) * GC * 8],
                                num_idxs=GC * P, num_idxs_reg=GC * P, elem_size=R2,
                            )
                            chunk2[c2] = g2
                        S = wpD.tile([P, P], BF16, tag="S2")
                        nc.vector.tensor_tensor(
                            out=S[:], in0=dstp[:, t:t + 1].to_broadcast([P, P]),
                            in1=iotaf[:], op=mybir.AluOpType.is_equal,
                        )
                        bp = (t % 3) * 32
                        dmat = psD2.tile([P, P], F32, tag="dmat2")
                        nc.tensor.matmul(
                            dmat[:], lhsT=ones1[bp:bp + 1, :],
                            rhs=dstf[bp:bp + 1, (t // 3) * P:(t // 3 + 1) * P],
                            start=True, stop=True,
                        )
                        ST = wpD.tile([P, P], BF16, tag="ST2")
                        nc.vector.tensor_tensor(
                            out=ST[:], in0=iotap[:, 0:1].to_broadcast([P, P]),
                            in1=dmat[:], op=mybir.AluOpType.is_equal,
                        )
                        ade2 = psE2.tile([P, 1], F32, tag="ade2")
                        nc.tensor.matmul(
                            ade2[:], lhsT=ST[:], rhs=ad2w[:, w:w + 1],
                            start=True, stop=True,
                        )
                        g2v = chunk2[c2]
                        esc2 = wpD.tile([P, 1], F32, tag="esc2")
                        nc.vector.tensor_add(
                            out=esc2[:], in0=g2v[:, slot2 * R2 + OUT_C: slot2 * R2 + OUT_C + 1],
                            in1=ade2[:],
                        )
                        elr2 = wpD.tile([P, 1], F32, tag="elr2")
                        nc.vector.scalar_tensor_tensor(
                            out=elr2[:], in0=esc2[:], scalar=NEG, in1=esc2[:],
                            op0=mybir.AluOpType.mult, op1=mybir.AluOpType.max,
                        )
                        rhs2 = wpD.tile([P, OC1], BF16, tag="rhs2")
                        nc.scalar.activation(rhs2[:, OUT_C:OC1], elr2[:], mybir.ActivationFunctionType.Exp)
                        nc.vector.tensor_mul(
                            out=rhs2[:, :OUT_C],
                            in0=g2v[:, slot2 * R2: slot2 * R2 + OUT_C],
                            in1=rhs2[:, OUT_C:OC1].to_broadcast([P, OUT_C]),
                        )
                        nc.tensor.matmul(
                            acc2[:], lhsT=S[:], rhs=rhs2[:],
                            start=(tl == 0), stop=(tl == TPW - 1),
                        )
                    zr2 = wpD.tile([P, 1], F32, tag="zr2")
                    nc.vector.reciprocal(zr2[:], acc2[:, OUT_C:OC1])
                    o2 = wpD.tile([P, OUT_C], F32, tag="o2")
                    nc.vector.tensor_mul(
                        out=o2[:], in0=acc2[:, :OUT_C],
                        in1=zr2[:].to_broadcast([P, OUT_C]),
                    )
                    nc.vector.tensor_add(out=o2[:], in0=o2[:], in1=b2m[:])
                    # log_softmax
                    mneg = wpD.tile([P, 1], F32, tag="mneg")
                    nc.vector.tensor_reduce(
                        out=mneg[:], in_=o2[:], axis=mybir.AxisListType.X,
                        op=mybir.AluOpType.max, negate=True,
                    )
                    ex = wpD.tile([P, OUT_C], F32, tag="ex")
                    ssum = wpD.tile([P, 1], F32, tag="ssum")
                    nc.scalar.activation(
                        ex[:], o2[:], mybir.ActivationFunctionType.Exp,
                        bias=mneg[:, 0:1], accum_out=ssum[:, 0:1],
                    )
                    lns = wpD.tile([P, 1], F32, tag="lns")
                    nc.scalar.activation(lns[:], ssum[:], mybir.ActivationFunctionType.Ln)
                    comb = wpD.tile([P, 1], F32, tag="comb")
                    nc.vector.tensor_sub(out=comb[:], in0=mneg[:], in1=lns[:])
                    fin = wpD.tile([P, OUT_C], F32, tag="fin")
                    nc.vector.tensor_scalar_add(out=fin[:], in0=o2[:], scalar1=comb[:, 0:1])
                    nc.sync.dma_start(out=out_d[w * P:(w + 1) * P, :], in_=fin[:])

    nc.compile()
    return nc


def _prepare(x, edge_index, w1, att_src1, att_dst1, b1, w2, att_src2, att_dst2, b2):
    x = np.asarray(x, dtype=np.float32)
    edge_index = np.asarray(edge_index)
    w1 = np.asarray(w1, dtype=np.float32)
    att_src1 = np.asarray(att_src1, dtype=np.float32)
    att_dst1 = np.asarray(att_dst1, dtype=np.float32)
    b1 = np.asarray(b1, dtype=np.float32)
    w2 = np.asarray(w2, dtype=np.float32)
    att_src2 = np.asarray(att_src2, dtype=np.float32)
    att_dst2 = np.asarray(att_dst2, dtype=np.float32)
    b2 = np.asarray(b2, dtype=np.float32)

    N, IN_C = x.shape
    H, C = att_src1.shape
    F1 = H * C
    OUT_C = w2.shape[1]
    assert N % NCORES == 0
    NPC = N // NCORES
    W = (NPC + P - 1) // P
    NPAD = W * P

    # ---- edges: append self-loops, sort by destination ----
    E = edge_index.shape[1]
    src = np.concatenate([edge_index[0].astype(np.int64), np.arange(N, dtype=np.int64)])
    dst = np.concatenate([edge_index[1].astype(np.int64), np.arange(N, dtype=np.int64)])
    order = np.argsort(dst, kind="stable")
    src, dst = src[order], dst[order]

    core_of = dst // NPC
    bounds = np.searchsorted(dst, np.arange(NCORES + 1) * NPC)
    win_of = (dst - core_of * NPC) // P

    # tiles per window (uniform across cores/windows for SPMD)
    TPW = 1
    counts = np.zeros((NCORES, W), np.int64)
    for cidx in range(NCORES):
        w_arr = win_of[bounds[cidx]:bounds[cidx + 1]]
        cnt = np.bincount(w_arr, minlength=W)
        counts[cidx] = cnt
    TPW = max(1, int(np.ceil(counts.max() / P)))
    T = W * TPW
    TG = ((T + 7) // 8) * 8

    # blocked node id inside the AllGather table
    blocked = (src // NPC) * NPAD + (src % NPC)
    assert NCORES * NPAD < 32768

    idx_arrs, dstp_arrs, dstf_arrs = [], [], []
    for cidx in range(NCORES):
        ids = np.zeros(TG * P, np.int16)          # padding gathers row 0
        dloc = np.full(T * P, 255.0, np.float32)  # padding -> no one-hot match
        s_c = blocked[bounds[cidx]:bounds[cidx + 1]]
        w_c = win_of[bounds[cidx]:bounds[cidx + 1]]
        d_c = dst[bounds[cidx]:bounds[cidx + 1]] - cidx * NPC
        wb = np.searchsorted(w_c, np.arange(W + 1))
        for w in range(W):
            n = wb[w + 1] - wb[w]
            base = w * TPW * P
            ids[base:base + n] = s_c[wb[w]:wb[w + 1]]
            dloc[base:base + n] = (d_c[wb[w]:wb[w + 1]] - w * P).astype(np.float32)
        # gather index layout (8-tile chunks): chunk c col-block [c*64, (c+1)*64),
        # within-chunk position i at [i%16, c*64 + i//16], replicated to 128 rows
        idx16 = np.zeros((16, TG * 8), np.int16)
        gpos = np.arange(TG * P)
        cc, ii = gpos // (8 * P), gpos % (8 * P)
        idx16[ii % 16, cc * 64 + ii // 16] = ids
        idx_arrs.append(np.tile(idx16, (8, 1)))
        pos = np.arange(T * P)
        # dstloc partition-major [128, T] and flat [8, T*16]
        dstp_arrs.append(_to_bf(dloc.reshape(T, P).T))
        dflat = np.zeros((P, ((T + 2) // 3) * P), np.float32)
        tt = np.arange(T * P) // P
        dflat[(tt % 3) * 32, (tt // 3) * P + pos % P] = dloc
        dstf_arrs.append(_to_bf(dflat))

    # ---- dense-layer host prep ----
    xT = np.zeros((IN_C, NCORES, NPAD), np.float32)
    xT[:, :, :NPC] = x.T.reshape(IN_C, NCORES, NPC)
    w1f = w1 + b1 * 0.0  # bias handled separately
    attsrc_mat = np.tile(att_src1.reshape(1, F1), (P, 1))
    attdst_mat = np.tile(att_dst1.reshape(1, F1), (P, 1))
    w2e = np.concatenate([w2, w2 @ att_src2.T, w2 @ att_dst2.T], axis=1)
    b1m = np.tile(b1.reshape(1, F1), (P, 1)).astype(np.float32)
    b2m = np.tile(b2.reshape(1, OUT_C), (P, 1)).astype(np.float32)
    iotaf = np.tile(np.arange(P, dtype=np.float32)[None, :], (P, 1))
    iotap = np.arange(P, dtype=np.float32)[:, None]

    nc = _build_program(N, IN_C, F1, H, C, OUT_C, NPAD, T, TPW, W, TG)

    in_maps = []
    for cidx in range(NCORES):
        in_maps.append({
            "xT": _to_bf(xT[:, cidx]),
            "w1b": _to_bf(w1f),
            "attsrc": _to_bf(attsrc_mat),
            "attdst": _to_bf(attdst_mat),
            "w2e": _to_bf(w2e),
            "b1m": b1m,
            "b2m": b2m,
            "idx": idx_arrs[cidx],
            "dstp": dstp_arrs[cidx],
            "dstf": dstf_arrs[cidx],
            "iotaf": _to_bf(iotaf),
            "iotap": _to_bf(iotap),
        })
    return nc, in_maps, NPC


def kernel(_trace=False, **inputs):
    nc, in_maps, NPC = _prepare(**inputs)
    res = run_bass_kernel_spmd(nc, in_maps, core_ids=list(range(NCORES)), trace=_trace)
    out = np.concatenate([res.results[cidx]["out"][:NPC] for cidx in range(NCORES)], axis=0)
    kernel.last_exec_time_ns = res.exec_time_ns
    kernel.last_res = res
    return out.astype(np.float32)


# revision 13
# speedup vs baseline: 1.4390x; 1.4390x over previous
"""Two-layer GAT on 8 Trainium2 NeuronCores.

Strategy: shard by destination-node range. Edges (with self-loops) are sorted
by destination on the host; core c owns destinations [c*NPC, (c+1)*NPC) and
computes those output rows entirely locally, so per-destination softmax and
aggregation need no cross-core reduction. The only collectives are two
AllGathers of the packed node features (h+a_s after layer-1 GEMM, h2+scores
after layer-2 GEMM) so every core can gather arbitrary source rows.

Per core, per layer the message passing runs over fixed-size edge tiles of
128 edges grouped into 128-destination windows:
  - dma_gather fetches packed [h | a_s] rows (768B) for 8-tile chunks
    (1024 indices per instruction - the ucode's hard cap),
  - a one-hot matrix S[e, d] built with is_equal (tensor_scalar, bf16 2x)
    scatters exp-weighted messages into PSUM via the TensorEngine,
  - S^T (is_equal against a host-shipped partition-replicated dst-local
    matrix) pulls a_d[dst] to edge slots via a small matmul,
  - attention scores, leaky-relu and exp are batched per window.
"""
import sys

sys.path.insert(0, "/opt/trn_rl_repo")
import numpy as np
import ml_dtypes

import concourse.bass as bass
import concourse.bacc as bacc
import concourse.mybir as mybir
import concourse.tile as tile
from concourse.bass_utils import run_bass_kernel_spmd
from concourse.masks import make_identity

BF16 = mybir.dt.bfloat16
F32 = mybir.dt.float32
I16 = mybir.dt.int16
nbf16 = ml_dtypes.bfloat16

NCORES = 8
NEG = 0.2
P = 128
GC = 8          # edge tiles per gather chunk (1024 idxs = ucode cap)


def _to_bf(a):
    return np.ascontiguousarray(np.asarray(a, dtype=np.float32).astype(nbf16))


def _build_program(IN_C, F1, H, C, OUT_C, NPAD, T, TPW, W, TG, add_b1, add_b2):
    KT1 = IN_C // P
    KT2 = F1 // P
    F1C = F1 // P
    OCE = OUT_C + 2            # gemm2 columns: [w2 | w2@as2 | w2@ad2]
    OC1 = OUT_C + 1            # layer-2 scatter rhs: [msg | exp]
    F1E = F1 + 2 * H           # gemm1 columns: [w1 | w1@As | w1@Ad]
    F1A = F1 + H               # packed row content: [h | a_s]
    F1R = ((F1A * 2 + 255) // 256) * 128   # gather row width (bf16), 256B mult
    R2 = 128                   # layer-2 packed row width (bf16) -> 256B
    NROWS = NCORES * NPAD

    nc = bacc.Bacc(num_devices=NCORES)

    xT_d = nc.dram_tensor("xT", [IN_C, NPAD], BF16, kind="ExternalInput")
    w1e_d = nc.dram_tensor("w1e", [IN_C, F1E], BF16, kind="ExternalInput")
    w2e_d = nc.dram_tensor("w2e", [F1, OCE], BF16, kind="ExternalInput")
    b1_d = nc.dram_tensor("b1m", [P, F1], F32, kind="ExternalInput")
    b2_d = nc.dram_tensor("b2m", [P, OUT_C], F32, kind="ExternalInput")
    idx_d = nc.dram_tensor("idx", [P, TG * 8], I16, kind="ExternalInput")
    dstp_d = nc.dram_tensor("dstp", [P, T], F32, kind="ExternalInput")
    dstm_d = nc.dram_tensor("dstm", [P, T * P], BF16, kind="ExternalInput")
    iotaf_d = nc.dram_tensor("iotaf", [P, P], BF16, kind="ExternalInput")
    iotap_d = nc.dram_tensor("iotap", [P, 1], F32, kind="ExternalInput")
    out_d = nc.dram_tensor("out", [NPAD, OUT_C], F32, kind="ExternalOutput")

    hloc_d = nc.dram_tensor("hloc", [NPAD, F1R], BF16)
    hfull_d = nc.dram_tensor("hfull", [NROWS, F1R], BF16, addr_space="Shared")
    h2loc_d = nc.dram_tensor("h2loc", [NPAD, R2], BF16)
    h2full_d = nc.dram_tensor("h2full", [NROWS, R2], BF16, addr_space="Shared")

    rg = [list(range(NCORES))]

    with tile.TileContext(nc) as tc:
        with (
            tc.tile_pool(name="const", bufs=1) as cp,
            tc.tile_pool(name="persist", bufs=1) as pp,
        ):
            iotaf = cp.tile([P, P], BF16)
            nc.sync.dma_start(out=iotaf[:], in_=iotaf_d[:, :])
            iotap = cp.tile([P, 1], F32)
            nc.sync.dma_start(out=iotap[:], in_=iotap_d[:, :])
            b1m = cp.tile([P, F1], F32)
            if add_b1:
                nc.sync.dma_start(out=b1m[:], in_=b1_d[:, :])
            b2m = cp.tile([P, OUT_C], F32)
            if add_b2:
                nc.sync.dma_start(out=b2m[:], in_=b2_d[:, :])
            w1sb = cp.tile([P, KT1 * F1E], BF16)
            for k in range(KT1):
                nc.sync.dma_start(out=w1sb[:, k * F1E:(k + 1) * F1E], in_=w1e_d[k * P:(k + 1) * P, :])
            w2esb = cp.tile([P, KT2 * OCE], BF16)
            for k in range(KT2):
                nc.sync.dma_start(out=w2esb[:, k * OCE:(k + 1) * OCE], in_=w2e_d[k * P:(k + 1) * P, :])
            idxsb = cp.tile([P, TG * 8], I16)
            nc.sync.dma_start(out=idxsb[:], in_=idx_d[:, :])
            dstp = cp.tile([P, T], F32)
            nc.sync.dma_start(out=dstp[:], in_=dstp_d[:, :])
            ident = cp.tile([P, P], BF16)
            make_identity(nc, ident[:])

            xT = pp.tile([P, KT1 * NPAD], BF16)
            for k in range(KT1):
                nc.sync.dma_start(out=xT[:, k * NPAD:(k + 1) * NPAD], in_=xT_d[k * P:(k + 1) * P, :])
            adw = pp.tile([P, W * H], BF16)
            ad2w = pp.tile([P, W], BF16)
            h1T = pp.tile([P, F1C * NPAD], BF16)
            asall = pp.tile([P, TG * H], BF16)   # per-edge a_s copied from gather chunks
            as2all = pp.tile([P, TG], BF16)      # per-edge a_s2 for layer 2

            # ---------------- Phase A: GEMM1 -> [h | a_s | a_d] ----------------
            with (
                tc.tile_pool(name="psA", bufs=2, space="PSUM") as psA,
                tc.tile_pool(name="wpA", bufs=3) as wpA,
            ):
                for m in range(W):
                    ph = psA.tile([P, F1E], F32, tag="ph")
                    for k in range(KT1):
                        nc.tensor.matmul(
                            ph[:],
                            lhsT=xT[:, k * NPAD + m * P: k * NPAD + (m + 1) * P],
                            rhs=w1sb[:, k * F1E:(k + 1) * F1E],
                            start=(k == 0), stop=(k == KT1 - 1),
                        )
                    nc.vector.tensor_copy(out=adw[:, m * H:(m + 1) * H], in_=ph[:, F1 + H:F1E])
                    hb = wpA.tile([P, F1A], BF16, tag="hb")
                    nc.vector.tensor_copy(out=hb[:], in_=ph[:, :F1A])
                    nc.sync.dma_start(out=hloc_d[m * P:(m + 1) * P, :F1A], in_=hb[:])

            nc.gpsimd.collective_compute(
                "AllGather", mybir.AluOpType.bypass, replica_groups=rg,
                ins=[hloc_d.ap()], outs=[hfull_d.ap()],
            )

            # ---------------- Phase B: layer-1 message passing ----------------
            with (
                tc.tile_pool(name="gpB", bufs=4) as gpB,
                tc.tile_pool(name="dmB", bufs=2) as dmB,
                tc.tile_pool(name="psAcc", bufs=2, space="PSUM") as psAcc,
                tc.tile_pool(name="psAde", bufs=2, space="PSUM") as psAde,
                tc.tile_pool(name="psT", bufs=2, space="PSUM") as psT,
                tc.tile_pool(name="wpB", bufs=2) as wpB,
                tc.tile_pool(name="wpBs", bufs=3) as wpBs,
            ):
                chunk_tiles = {}

                def get_chunk(c):
                    if c in chunk_tiles:
                        return chunk_tiles[c]
                    gt = gpB.tile([P, GC * F1R], BF16, tag="gath")
                    nc.gpsimd.dma_gather(
                        out_ap=gt[:].rearrange("p (t f) -> p t f", f=F1R),
                        in_ap=hfull_d.ap(),
                        idxs_ap=idxsb[:, c * GC * 8:(c + 1) * GC * 8],
                        num_idxs=GC * P, num_idxs_reg=GC * P, elem_size=F1R,
                    )
                    # contiguous per-edge a_s for batched score math
                    nc.vector.tensor_copy(
                        out=asall[:, c * GC * H:(c + 1) * GC * H].rearrange(
                            "p (t h) -> p t h", h=H),
                        in_=gt[:].rearrange("p (t f) -> p t f", f=F1R)[:, :, F1:F1 + H],
                    )
                    chunk_tiles[c] = gt
                    return gt

                for w in range(W):
                    acc = psAcc.tile([P, F1A], F32, tag="acc")
                    adew = psAde.tile([P, TPW * H], F32, tag="adew")
                    Swin = wpB.tile([P, TPW * P], BF16, tag="Swin")
                    dstm = dmB.tile([P, TPW * P], BF16, tag="dstm")
                    nc.sync.dma_start(
                        out=dstm[:], in_=dstm_d[:, w * TPW * P:(w + 1) * TPW * P])
                    STw = wpB.tile([P, TPW * P], BF16, tag="STw")
                    nc.vector.tensor_scalar(
                        out=STw[:], in0=dstm[:], scalar1=iotap[:, 0:1], scalar2=None,
                        op0=mybir.AluOpType.is_equal,
                    )
                    for tl in range(TPW):
                        t = w * TPW + tl
                        get_chunk(t // GC)
                        nc.vector.tensor_scalar(
                            out=Swin[:, tl * P:(tl + 1) * P], in0=iotaf[:],
                            scalar1=dstp[:, t:t + 1], scalar2=None,
                            op0=mybir.AluOpType.is_equal,
                        )
                        nc.tensor.matmul(
                            adew[:, tl * H:(tl + 1) * H],
                            lhsT=STw[:, tl * P:(tl + 1) * P],
                            rhs=adw[:, w * H:(w + 1) * H],
                            start=True, stop=True,
                        )
                    escw = wpBs.tile([P, TPW * H], F32, tag="escw")
                    nc.vector.tensor_add(
                        out=escw[:], in0=adew[:],
                        in1=asall[:, w * TPW * H:(w + 1) * TPW * H])
                    elrw = wpBs.tile([P, TPW * H], F32, tag="elrw")
                    nc.vector.scalar_tensor_tensor(
                        out=elrw[:], in0=escw[:], scalar=NEG, in1=escw[:],
                        op0=mybir.AluOpType.mult, op1=mybir.AluOpType.max,
                    )
                    expw = wpBs.tile([P, TPW * H], BF16, tag="expw")
                    nc.scalar.activation(expw[:], elrw[:], mybir.ActivationFunctionType.Exp)
                    for tl in range(TPW):
                        t = w * TPW + tl
                        c, slot = divmod(t, GC)
                        gt = chunk_tiles[c]
                        hsrc = gt[:, slot * F1R: slot * F1R + F1]
                        rhs = wpBs.tile([P, F1A], BF16, tag="rhs")
                        nc.vector.tensor_mul(
                            out=rhs[:, :F1].rearrange("p (h c) -> p h c", c=C),
                            in0=hsrc.rearrange("p (h c) -> p h c", c=C),
                            in1=expw[:, tl * H:(tl + 1) * H]
                                .rearrange("p (h c) -> p h c", c=1).to_broadcast([P, H, C]),
                        )
                        nc.vector.tensor_copy(
                            out=rhs[:, F1:F1A], in_=expw[:, tl * H:(tl + 1) * H])
                        nc.tensor.matmul(
                            acc[:], lhsT=Swin[:, tl * P:(tl + 1) * P], rhs=rhs[:],
                            start=(tl == 0), stop=(tl == TPW - 1),
                        )
                    # finalize window: out1 = acc/z (+b1), ELU, transpose
                    zr = wpBs.tile([P, H], F32, tag="zr")
                    nc.vector.reciprocal(zr[:], acc[:, F1:F1A])
                    o1 = wpBs.tile([P, F1], F32, tag="o1")
                    nc.vector.tensor_mul(
                        out=o1[:].rearrange("p (h c) -> p h c", c=C),
                        in0=acc[:, :F1].rearrange("p (h c) -> p h c", c=C),
                        in1=zr[:].rearrange("p (h c) -> p h c", c=1).to_broadcast([P, H, C]),
                    )
                    if add_b1:
                        nc.vector.tensor_add(out=o1[:], in0=o1[:], in1=b1m[:])
                    mn = wpBs.tile([P, F1], F32, tag="mn")
                    nc.vector.tensor_scalar_min(out=mn[:], in0=o1[:], scalar1=0.0)
                    em = wpBs.tile([P, F1], F32, tag="em")
                    nc.scalar.activation(em[:], mn[:], mybir.ActivationFunctionType.Exp)
                    mx = wpBs.tile([P, F1], F32, tag="mx")
                    nc.vector.tensor_scalar_max(out=mx[:], in0=o1[:], scalar1=0.0)
                    h1 = wpBs.tile([P, F1], BF16, tag="h1")
                    nc.vector.scalar_tensor_tensor(
                        out=h1[:], in0=em[:], scalar=-1.0, in1=mx[:],
                        op0=mybir.AluOpType.add, op1=mybir.AluOpType.add,
                    )
                    for fc in range(F1C):
                        tp = psT.tile([P, P], BF16, tag="tp")
                        nc.tensor.transpose(tp[:], h1[:, fc * P:(fc + 1) * P], ident[:])
                        nc.vector.tensor_copy(
                            out=h1T[:, fc * NPAD + w * P: fc * NPAD + (w + 1) * P], in_=tp[:],
                        )

            # ---------------- Phase C: GEMM2 + pack ----------------
            with (
                tc.tile_pool(name="psC", bufs=2, space="PSUM") as psC,
                tc.tile_pool(name="wpC", bufs=3) as wpC,
            ):
                for m in range(W):
                    p2 = psC.tile([P, OCE], F32, tag="p2")
                    for k in range(KT2):
                        nc.tensor.matmul(
                            p2[:],
                            lhsT=h1T[:, k * NPAD + m * P: k * NPAD + (m + 1) * P],
                            rhs=w2esb[:, k * OCE:(k + 1) * OCE],
                            start=(k == 0), stop=(k == KT2 - 1),
                        )
                    h2r = wpC.tile([P, R2], BF16, tag="h2r")
                    nc.vector.memset(h2r[:, OCE:], 0.0)
                    nc.vector.tensor_copy(out=h2r[:, :OCE], in_=p2[:])
                    nc.vector.tensor_copy(out=ad2w[:, m:m + 1], in_=p2[:, OCE - 1:OCE])
                    nc.sync.dma_start(out=h2loc_d[m * P:(m + 1) * P, :], in_=h2r[:])

            nc.gpsimd.collective_compute(
                "AllGather", mybir.AluOpType.bypass, replica_groups=rg,
                ins=[h2loc_d.ap()], outs=[h2full_d.ap()],
            )

            # ---------------- Phase D: layer-2 message passing ----------------
            with (
                tc.tile_pool(name="gpD", bufs=4) as gpD,
                tc.tile_pool(name="dmD", bufs=2) as dmD,
                tc.tile_pool(name="psAcc2", bufs=2, space="PSUM") as psAcc2,
                tc.tile_pool(name="psAde2", bufs=2, space="PSUM") as psAde2,
                tc.tile_pool(name="wpD", bufs=2) as wpD,
                tc.tile_pool(name="wpDs", bufs=3) as wpDs,
            ):
                chunk2 = {}

                def get_chunk2(c):
                    if c in chunk2:
                        return chunk2[c]
                    g2 = gpD.tile([P, GC * R2], BF16, tag="gath2")
                    nc.gpsimd.dma_gather(
                        out_ap=g2[:].rearrange("p (t f) -> p t f", f=R2),
                        in_ap=h2full_d.ap(),
                        idxs_ap=idxsb[:, c * GC * 8:(c + 1) * GC * 8],
                        num_idxs=GC * P, num_idxs_reg=GC * P, elem_size=R2,
                    )
                    nc.vector.tensor_copy(
                        out=as2all[:, c * GC:(c + 1) * GC].rearrange(
                            "p (t o) -> p t o", o=1),
                        in_=g2[:].rearrange("p (t f) -> p t f", f=R2)[:, :, OUT_C:OUT_C + 1],
                    )
                    chunk2[c] = g2
                    return g2

                for w in range(W):
                    acc2 = psAcc2.tile([P, OC1], F32, tag="acc2")
                    adew2 = psAde2.tile([P, TPW], F32, tag="adew2")
                    Swin2 = wpD.tile([P, TPW * P], BF16, tag="Swin2")
                    dstm2 = dmD.tile([P, TPW * P], BF16, tag="dstm2")
                    nc.sync.dma_start(
                        out=dstm2[:], in_=dstm_d[:, w * TPW * P:(w + 1) * TPW * P])
                    STw2 = wpD.tile([P, TPW * P], BF16, tag="STw2")
                    nc.vector.tensor_scalar(
                        out=STw2[:], in0=dstm2[:], scalar1=iotap[:, 0:1], scalar2=None,
                        op0=mybir.AluOpType.is_equal,
                    )
                    for tl in range(TPW):
                        t = w * TPW + tl
                        get_chunk2(t // GC)
                        nc.vector.tensor_scalar(
                            out=Swin2[:, tl * P:(tl + 1) * P], in0=iotaf[:],
                            scalar1=dstp[:, t:t + 1], scalar2=None,
                            op0=mybir.AluOpType.is_equal,
                        )
                        nc.tensor.matmul(
                            adew2[:, tl:tl + 1],
                            lhsT=STw2[:, tl * P:(tl + 1) * P],
                            rhs=ad2w[:, w:w + 1],
                            start=True, stop=True,
                        )
                    # esc2 = a_s2[src] + ade2 : a_s2 is col OUT_C of gathered rows
                    escw2 = wpDs.tile([P, TPW], F32, tag="escw2")
                    nc.vector.tensor_add(
                        out=escw2[:], in0=adew2[:],
                        in1=as2all[:, w * TPW:(w + 1) * TPW])
                    elrw2 = wpDs.tile([P, TPW], F32, tag="elrw2")
                    nc.vector.scalar_tensor_tensor(
                        out=elrw2[:], in0=escw2[:], scalar=NEG, in1=escw2[:],
                        op0=mybir.AluOpType.mult, op1=mybir.AluOpType.max,
                    )
                    expw2 = wpDs.tile([P, TPW], BF16, tag="expw2")
                    nc.scalar.activation(expw2[:], elrw2[:], mybir.ActivationFunctionType.Exp)
                    for tl in range(TPW):
                        t = w * TPW + tl
                        c2, slot2 = divmod(t, GC)
                        g2v = chunk2[c2]
                        rhs2 = wpDs.tile([P, OC1], BF16, tag="rhs2")
                        nc.vector.tensor_mul(
                            out=rhs2[:, :OUT_C],
                            in0=g2v[:, slot2 * R2: slot2 * R2 + OUT_C],
                            in1=expw2[:, tl:tl + 1].to_broadcast([P, OUT_C]),
                        )
                        nc.vector.tensor_copy(
                            out=rhs2[:, OUT_C:OC1], in_=expw2[:, tl:tl + 1])
                        nc.tensor.matmul(
                            acc2[:], lhsT=Swin2[:, tl * P:(tl + 1) * P], rhs=rhs2[:],
                            start=(tl == 0), stop=(tl == TPW - 1),
                        )
                    zr2 = wpDs.tile([P, 1], F32, tag="zr2")
                    nc.vector.reciprocal(zr2[:], acc2[:, OUT_C:OC1])
                    o2 = wpDs.tile([P, OUT_C], F32, tag="o2")
                    nc.vector.tensor_mul(
                        out=o2[:], in0=acc2[:, :OUT_C],
                        in1=zr2[:].to_broadcast([P, OUT_C]),
                    )
                    if add_b2:
                        nc.vector.tensor_add(out=o2[:], in0=o2[:], in1=b2m[:])
                    mneg = wpDs.tile([P, 1], F32, tag="mneg")
                    nc.vector.tensor_reduce(
                        out=mneg[:], in_=o2[:], axis=mybir.AxisListType.X,
                        op=mybir.AluOpType.max, negate=True,
                    )
                    ex = wpDs.tile([P, OUT_C], F32, tag="ex")
                    ssum = wpDs.tile([P, 1], F32, tag="ssum")
                    nc.scalar.activation(
                        ex[:], o2[:], mybir.ActivationFunctionType.Exp,
                        bias=mneg[:, 0:1], accum_out=ssum[:, 0:1],
                    )
                    lns = wpDs.tile([P, 1], F32, tag="lns")
                    nc.scalar.activation(lns[:], ssum[:], mybir.ActivationFunctionType.Ln)
                    comb = wpDs.tile([P, 1], F32, tag="comb")
                    nc.vector.tensor_sub(out=comb[:], in0=mneg[:], in1=lns[:])
                    fin = wpDs.tile([P, OUT_C], F32, tag="fin")
                    nc.vector.tensor_scalar_add(out=fin[:], in0=o2[:], scalar1=comb[:, 0:1])
                    nc.sync.dma_start(out=out_d[w * P:(w + 1) * P, :], in_=fin[:])

    nc.compile()
    return nc


def _prepare(x, edge_index, w1, att_src1, att_dst1, b1, w2, att_src2, att_dst2, b2):
    x = np.asarray(x, dtype=np.float32)
    edge_index = np.asarray(edge_index)
    w1 = np.asarray(w1, dtype=np.float32)
    att_src1 = np.asarray(att_src1, dtype=np.float32)
    att_dst1 = np.asarray(att_dst1, dtype=np.float32)
    b1 = np.asarray(b1, dtype=np.float32)
    w2 = np.asarray(w2, dtype=np.float32)
    att_src2 = np.asarray(att_src2, dtype=np.float32)
    att_dst2 = np.asarray(att_dst2, dtype=np.float32)
    b2 = np.asarray(b2, dtype=np.float32)

    N, IN_C = x.shape
    H, C = att_src1.shape
    F1 = H * C
    OUT_C = w2.shape[1]
    assert N % NCORES == 0
    NPC = N // NCORES
    W = (NPC + P - 1) // P
    NPAD = W * P

    # ---- edges: append self-loops, sort by destination ----
    src = np.concatenate([edge_index[0].astype(np.int64), np.arange(N, dtype=np.int64)])
    dst = np.concatenate([edge_index[1].astype(np.int64), np.arange(N, dtype=np.int64)])
    order = np.argsort(dst, kind="stable")
    src, dst = src[order], dst[order]

    core_of = dst // NPC
    bounds = np.searchsorted(dst, np.arange(NCORES + 1) * NPC)
    win_of = (dst - core_of * NPC) // P

    counts = np.zeros((NCORES, W), np.int64)
    for cidx in range(NCORES):
        w_arr = win_of[bounds[cidx]:bounds[cidx + 1]]
        counts[cidx] = np.bincount(w_arr, minlength=W)
    TPW = max(1, int(np.ceil(counts.max() / P)))
    T = W * TPW
    TG = ((T + GC - 1) // GC) * GC

    blocked = (src // NPC) * NPAD + (src % NPC)
    assert NCORES * NPAD < 32768

    idx_arrs, dstp_arrs, dstm_arrs = [], [], []
    for cidx in range(NCORES):
        ids = np.zeros(TG * P, np.int16)          # padding gathers row 0
        dloc = np.full(T * P, 255.0, np.float32)  # padding -> no one-hot match
        s_c = blocked[bounds[cidx]:bounds[cidx + 1]]
        w_c = win_of[bounds[cidx]:bounds[cidx + 1]]
        d_c = dst[bounds[cidx]:bounds[cidx + 1]] - cidx * NPC
        wb = np.searchsorted(w_c, np.arange(W + 1))
        for w in range(W):
            n = wb[w + 1] - wb[w]
            base = w * TPW * P
            ids[base:base + n] = s_c[wb[w]:wb[w + 1]]
            dloc[base:base + n] = (d_c[wb[w]:wb[w + 1]] - w * P).astype(np.float32)
        # gather index layout (GC-tile chunks): chunk c col-block [c*64,(c+1)*64)
        idx16 = np.zeros((16, TG * 8), np.int16)
        gpos = np.arange(TG * P)
        cc, ii = gpos // (GC * P), gpos % (GC * P)
        idx16[ii % 16, cc * 64 + ii // 16] = ids
        idx_arrs.append(np.tile(idx16, (8, 1)))
        dstp_arrs.append(np.ascontiguousarray(dloc.reshape(T, P).T))
        # partition-replicated dst-locals for the S^T build
        dstm_arrs.append(_to_bf(np.tile(dloc[None, :], (P, 1))))

    xT = np.zeros((IN_C, NCORES, NPAD), np.float32)
    xT[:, :, :NPC] = x.T.reshape(IN_C, NCORES, NPC)
    Asrc = np.zeros((F1, H), np.float32)
    Adst = np.zeros((F1, H), np.float32)
    for h in range(H):
        Asrc[h * C:(h + 1) * C, h] = att_src1[h]
        Adst[h * C:(h + 1) * C, h] = att_dst1[h]
    w1e = np.concatenate([w1, w1 @ Asrc, w1 @ Adst], axis=1)
    w2e = np.concatenate([w2, w2 @ att_src2.T, w2 @ att_dst2.T], axis=1)
    b1m = np.tile(b1.reshape(1, F1), (P, 1)).astype(np.float32)
    b2m = np.tile(b2.reshape(1, OUT_C), (P, 1)).astype(np.float32)
    iotaf = np.tile(np.arange(P, dtype=np.float32)[None, :], (P, 1))
    iotap = np.arange(P, dtype=np.float32)[:, None]
    add_b1 = bool(np.any(b1))
    add_b2 = bool(np.any(b2))

    nc = _build_program(IN_C, F1, H, C, OUT_C, NPAD, T, TPW, W, TG, add_b1, add_b2)

    in_maps = []
    for cidx in range(NCORES):
        in_maps.append({
            "xT": _to_bf(xT[:, cidx]),
            "w1e": _to_bf(w1e),
            "w2e": _to_bf(w2e),
            "b1m": b1m,
            "b2m": b2m,
            "idx": idx_arrs[cidx],
            "dstp": dstp_arrs[cidx],
            "dstm": dstm_arrs[cidx],
            "iotaf": _to_bf(iotaf),
            "iotap": iotap.astype(np.float32),
        })
    return nc, in_maps, NPC


def kernel(_trace=False, **inputs):
    nc, in_maps, NPC = _prepare(**inputs)
    res = run_bass_kernel_spmd(nc, in_maps, core_ids=list(range(NCORES)), trace=_trace)
    out = np.concatenate([res.results[cidx]["out"][:NPC] for cidx in range(NCORES)], axis=0)
    kernel.last_exec_time_ns = res.exec_time_ns
    kernel.last_res = res
    return out.astype(np.float32)
